# revision 57
# baseline (speedup 1.0000x reference)
"""Trainium2 Bass kernel for DifferentialMultiHeadSelfAttention.

Sharding: 16 heads -> 8 cores (2 heads/core, tensor parallel). Everything up
to the output Linear is head-local (GroupNorm has num_groups == n_heads, so
each head's 64 channels normalize independently). Per-head AllToAlls exchange
normalized channel slices (head-0's collective hides behind head-1's
attention), then each core computes a 256-row slice of the output Linear.
The host concatenates the slices.

Key scheduling points:
- score matmuls for the two differential sub-spaces are K=64 row-group pairs
  run concurrently on the PE via tile_position (0,0)/(64,0)
- o^T transposes happen per-sv during attention (fp16), so the head tail is
  only the GroupNorm affine + a2a DMAs
- the head-0 half of the output Linear (K=64 chains) runs inside the head-1
  AllToAll window, keeping the PE warm; head-1's half lands on the same PSUM
  accumulation afterward

Shapes (hardcoded): B=1, S=2048, E=1024, H=16, DH=64.
"""
import numpy as np

from concourse import bacc, mybir, tile
from concourse.bass_utils import run_bass_kernel_spmd

# Pin all ScalarE activations to the one table set that covers every function
# used here (Exp, Ln, Square, Copy, Identity) so the table never reloads.
_orig_gat = bacc.get_activation_tables


def _single_set_tables(arch):
    t = _orig_gat(arch)
    target = t.get("natural_log_exp_and_others")
    if target is None:
        return t
    out = {}
    for name, fns in t.items():
        if name == "natural_log_exp_and_others":
            out[name] = fns
        else:
            kept = {f for f in fns if f not in target}
            out[name] = kept if kept else set(fns)
    return out


bacc.get_activation_tables = _single_set_tables

dt = mybir.dt

NCORES = 8
S = 2048
E = 1024
H = 16
DH = 64
HLOC = H // NCORES          # heads per core = 2
CLOC = HLOC * DH            # channels per core = 128
SLOC = S // NCORES          # output rows per core = 256
NT = S // 128               # 16 t-tiles
EPS = 1e-5

_PROGRAM_CACHE = {}


def _classify_mask(mask):
    """mask[s, t] bool, True = masked. Returns per-(t_tile, s_tile) state:
    0 = fully masked (skip), 1 = fully unmasked, 2 = needs mask multiply;
    plus the s-range start per t-tile and the keep (0/1) tiles in [t, s]
    layout for the state-2 blocks."""
    m = mask.reshape(NT, 128, NT, 128)  # [s_tile, s_in, t_tile, t_in]
    state = np.empty((NT, NT), dtype=np.int32)  # [t_tile, s_tile]
    keep_tiles = []
    tile_idx = -np.ones((NT, NT), dtype=np.int32)
    for t in range(NT):
        for s in range(NT):
            blk = m[s, :, t, :]  # [s_in, t_in]
            if blk.all():
                state[t, s] = 0
            elif not blk.any():
                state[t, s] = 1
            else:
                state[t, s] = 2
                tile_idx[t, s] = len(keep_tiles)
                keep_tiles.append((~blk.T).astype(np.float16))  # [t_in, s_in]
    s0 = np.full(NT, NT, dtype=np.int32)
    for t in range(NT):
        act = np.nonzero(state[t] != 0)[0]
        if len(act):
            s0[t] = act[0]
            for s in range(act[0], NT):
                if state[t, s] == 0:
                    state[t, s] = 2
                    tile_idx[t, s] = len(keep_tiles)
                    keep_tiles.append(np.zeros((128, 128), dtype=np.float16))
    if not keep_tiles:
        keep_tiles.append(np.zeros((128, 128), dtype=np.float16))
    return state, s0, tile_idx, np.stack(keep_tiles)


def _build_program(state, s0, tile_idx, n_keep):
    nc = bacc.Bacc(None, num_devices=NCORES)

    # ---- external I/O (16-bit operands for all PE inputs) ----
    XT = nc.dram_tensor("xt", [128, 8 * S], dt.float16, kind="ExternalInput")
    WQK = nc.dram_tensor("wqk", [128, 4 * 8 * 128], dt.float16,
                         kind="ExternalInput")
    WV = nc.dram_tensor("wv", [128, 8 * 128], dt.float16, kind="ExternalInput")
    BLOB = nc.dram_tensor("blob", [128, 11], dt.float32, kind="ExternalInput")
    KEEP = nc.dram_tensor("keep", [128, n_keep * 256], dt.float16,
                          kind="ExternalInput")
    EYE = nc.dram_tensor("eye", [128, 128], dt.float16, kind="ExternalInput")
    WO = nc.dram_tensor("wo", [128, 8 * E], dt.float16, kind="ExternalInput")
    B16 = nc.dram_tensor("b16", [1, 128 + E], dt.float16, kind="ExternalInput")
    OUT = nc.dram_tensor("out_slice", [SLOC, E], dt.float32, kind="ExternalOutput")

    # internal DRAM: per-head AllToAll of normalized channel slices (fp16)
    a2a_in = [nc.dram_tensor(f"a2a_in{h}", [NCORES * DH, SLOC], dt.float16)
              for h in range(HLOC)]
    a2a_out = [nc.dram_tensor(f"a2a_out{h}", [NCORES * DH, SLOC], dt.float16)
               for h in range(HLOC)]
    groups = [list(range(NCORES))]

    Exp = mybir.ActivationFunctionType.Exp
    ADD = mybir.AluOpType.add
    SUB = mybir.AluOpType.subtract
    MUL = mybir.AluOpType.mult

    with tile.TileContext(nc) as tc:
        with tc.tile_pool(name="consts", bufs=1) as consts, \
             tc.tile_pool(name="qk", bufs=1) as qkp, \
             tc.tile_pool(name="vaug", bufs=1) as vaugp, \
             tc.tile_pool(name="oboth", bufs=1) as obothp:

            # ---- constants: one packed f32 blob + one fp16 blob ----
            blob = consts.tile([128, 11], dt.float32, tag="blob")
            nc.sync.dma_start(out=blob[:], in_=BLOB[:])
            bqk = blob[:, 0:4]
            bv = blob[:, 4:5]
            lamn = blob[:, 5:7]
            gnw2 = blob[:, 7:9]      # rows 64:128 duplicate 0:64 (sv-pair layout)
            gnb2 = blob[:, 9:11]
            b16 = consts.tile([1, 128 + E], dt.float16, tag="b16")
            nc.sync.dma_start(out=b16[:], in_=B16[:])
            ones1 = b16[:, 0:128]
            bo = b16[:, 128:128 + E]
            ones128 = consts.tile([1, 128], dt.float32, tag="ones128")
            nc.vector.memset(ones128[:], 1.0)
            ones_col = consts.tile([128, 1], dt.float32, tag="ones_col")
            nc.vector.memset(ones_col[:], 1.0)
            epsc = consts.tile([1, 1], dt.float32, tag="epsc")
            nc.vector.memset(epsc[:], EPS)
            # warm the activation table set at program start (hidden under
            # the input DMAs) so the first real Exp doesn't pay the load
            actwarm = consts.tile([1, 1], dt.float32, tag="actwarm")
            nc.scalar.activation(actwarm[:], blob[0:1, 0:1], Exp, scale=0.0)
            eye = consts.tile([128, 128], dt.float16, tag="eye")
            keep16 = consts.tile([128, n_keep, 2, 128], dt.float16, tag="keep16")
            wo = consts.tile([128, HLOC, 4, E], dt.float16, tag="wo")
            bo_bc = consts.tile([128, E], dt.float32, tag="bo_bc")

            # persistent activation tiles
            qk_sb = [qkp.tile([128, S], dt.float16, tag=f"qk{j}", name=f"qk{j}")
                     for j in range(4)]
            v_aug = [vaugp.tile([128, NT, 65], dt.float16, tag=f"va{h}", name=f"va{h}")
                     for h in range(HLOC)]
            stat4 = obothp.tile([128, 4], dt.float32, tag="stat4")
            ssum = obothp.tile([128, HLOC, NT], dt.float32, tag="ssum")
            ssq = obothp.tile([128, HLOC, NT], dt.float32, tag="ssq")
            # sv-pair layout: partitions 0:64 = channels of the even sv of a
            # pair, 64:128 = channels of the odd sv; free = (pair, s_local)
            oT2 = obothp.tile([128, 8, 128], dt.float16, tag="oT2")
            xn_h = [obothp.tile([128, 8, 128], dt.float16, tag=f"xnh{h}",
                                name=f"xnh{h}") for h in range(HLOC)]
            # xa: partitions = [64ch of even j | 64ch of odd j] per (head, jp);
            # separate tiles per head so head-0's Linear half has no false
            # dependency on head-1's post-collective DMAs
            xa = [obothp.tile([128, 4, SLOC], dt.float16, tag=f"xa{h}",
                              name=f"xa{h}") for h in range(HLOC)]

            # ================= phase 1: projections =================
            # operand tiles live in the persistent pool: the j2/j3 chains and
            # head-1 v_aug transposes are deferred into head-0's attention
            # loop so the ScalarE Exp stream starts ~25us earlier
            with tc.tile_pool(name="proj", bufs=1) as projp:
                wqk = obothp.tile([128, 4, 8, 128], dt.float16, tag="wqk")
                xt = obothp.tile([128, 8, S], dt.float16, tag="xt")
                # first compute chunk's operands first: wqk j=0, then the
                # sc=0 512-col strip of every e-chunk of x
                # spread the initial loads over issue queues (each dma_start
                # costs ~600ns of sequencer time); keep the heavy weight
                # blocks off the queues that feed the first compute chunks
                nc.sync.dma_start(
                    out=wqk[:, 0:1, :, :],
                    in_=WQK[:, 0:1024].rearrange("p (j q) -> p j q", j=1))
                wv = obothp.tile([128, 8, 128], dt.float16, tag="wv")
                qs = [nc.sync, nc.scalar, nc.gpsimd]
                for e in range(8):
                    qs[e % 3].dma_start(out=xt[:, e, 0:512],
                                        in_=XT[:, S * e:S * e + 512])
                nc.sync.dma_start(
                    out=wqk[:, 1:2, :, :],
                    in_=WQK[:, 1024:2048].rearrange("p (j q) -> p j q", j=1))
                nc.scalar.dma_start(out=wv[:], in_=WV[:])
                for e in range(8):
                    for half in range(2):
                        qs[(e + half) % 3].dma_start(
                            out=xt[:, e, 512 + 768 * half:512 + 768 * (half + 1)],
                            in_=XT[:, S * e + 512 + 768 * half:
                                   S * e + 512 + 768 * (half + 1)])
                nc.gpsimd.dma_start(
                    out=wqk[:, 2:4, :, :],
                    in_=WQK[:, 2048:4096].rearrange("p (j q) -> p j q", j=2))
                # bulky constants after the projection operands
                nc.gpsimd.dma_start(out=keep16[:], in_=KEEP[:].rearrange(
                    "p (k m q) -> p k m q", k=n_keep, m=2))
                nc.sync.dma_start(out=eye[:], in_=EYE[:])
                vT = obothp.tile([128, S], dt.float16, tag="vT")

                with tc.tile_pool(name="proj_ps", bufs=4, space="PSUM") as proj_ps:
                    # bias broadcast row first: needs only the tiny b16 DMA,
                    # warms the PE while the xt strips are still loading
                    for ec in range(2):
                        psb = proj_ps.tile([128, 512], dt.float32, tag="pps",
                                           name="psb")
                        nc.tensor.matmul(psb[:], ones1[:],
                                         bo[:, 512 * ec:512 * (ec + 1)],
                                         start=True, stop=True)
                        nc.vector.tensor_copy(bo_bc[:, 512 * ec:512 * (ec + 1)],
                                              psb[:])
                    for sc in range(4):
                        for j in range(2):  # qpair-h0, kpair-h0 (j2/j3 deferred)
                            ps = proj_ps.tile([128, 512], dt.float32, tag="pps")
                            for e in range(8):
                                nc.tensor.matmul(ps[:], wqk[:, j, e, :],
                                                 xt[:, e, 512 * sc:512 * (sc + 1)],
                                                 start=(e == 0), stop=(e == 7))
                            nc.vector.tensor_scalar(
                                qk_sb[j][:, 512 * sc:512 * (sc + 1)],
                                ps[:], bqk[:, j:j + 1], None, ADD)
                        ps = proj_ps.tile([128, 512], dt.float32, tag="pps")
                        for e in range(8):
                            nc.tensor.matmul(ps[:], wv[:, e, :],
                                             xt[:, e, 512 * sc:512 * (sc + 1)],
                                             start=(e == 0), stop=(e == 7))
                        nc.vector.tensor_scalar(vT[:, 512 * sc:512 * (sc + 1)],
                                                ps[:], bv[:], None, ADD)


                    # ---- phase 2: transpose vT -> v_aug, both heads in one
                    # [128,128] transpose per t (cols 0:64 = h0, 64:128 = h1)
                    with tc.tile_pool(name="vt_ps", bufs=4, space="PSUM") as vt_ps:
                        for t in range(NT):
                            nc.vector.memset(v_aug[0][:, t, 64:65], 1.0)
                            nc.vector.memset(v_aug[1][:, t, 64:65], 1.0)
                            ps = vt_ps.tile([128, 128], dt.float16, tag="vtps")
                            nc.tensor.transpose(
                                ps[:], vT[:, 128 * t:128 * (t + 1)], eye[:])
                            nc.vector.tensor_copy(v_aug[0][:, t, 0:64],
                                                  ps[:, 0:64])
                            nc.vector.tensor_copy(v_aug[1][:, t, 0:64],
                                                  ps[:, 64:128])

            # load wo during the attention phase (off the critical DMA path)
            nc.sync.dma_start(out=wo[:], in_=WO[:])

            # ================= phase 3: attention =================
            with tc.tile_pool(name="exp", bufs=1) as expp, \
                 tc.tile_pool(name="rr", bufs=8) as rrp, \
                 tc.tile_pool(name="ob", bufs=4) as obp, \
                 tc.tile_pool(name="sc_ps", bufs=2, space="PSUM") as sc_ps, \
                 tc.tile_pool(name="xp_ps", bufs=1, space="PSUM") as xp_ps, \
                 tc.tile_pool(name="o_ps", bufs=3, space="PSUM") as o_ps:
                for h in range(HLOC):
                    qT = qk_sb[2 * h]      # [0:64]=sub-1 dims, [64:128]=sub-2
                    kT = qk_sb[2 * h + 1]
                    # o-chain schedule: sv is ready once its last t-tile's exp
                    # exists; emit it one t-iteration later so the PE queue
                    # never blocks on the current tile's Exp (software
                    # pipelining of scores vs o-chains)
                    sv_ready = {}
                    for sv in range(NT):
                        ts = [tt for tt in range(NT)
                              if s0[tt] <= sv and state[tt, sv] != 0]
                        if ts:
                            sv_ready.setdefault(max(ts), []).append((sv, ts))
                    exp_tm = [[None] * NT, [None] * NT]
                    pair_tiles = {}
                    pair_count = {}

                    def flush_pair(pi):
                        # transpose a completed sv-pair: [128 s, 2*64] ->
                        # [(parity, ch) 128, 128 s] in one PE transpose
                        pst = xp_ps.tile([128, 128], dt.float16, tag="xpps",
                                         name="pst")
                        nc.tensor.transpose(
                            pst[:],
                            pair_tiles[pi][:].rearrange("p a b -> p (a b)"),
                            eye[:])
                        nc.vector.tensor_copy(oT2[:, pi, :], pst[:])

                    def emit_ochains(t):
                        for sv, ts in sv_ready.get(t, []):
                            ops = o_ps.tile([128, 2, 128], dt.float32, tag="ops")
                            for m in range(2):
                                for i, tt in enumerate(ts):
                                    b = 128 * int(s0[tt])
                                    nc.tensor.matmul(
                                        ops[:, m, 0:65],
                                        exp_tm[m][tt][:, 128 * sv - b:
                                                      128 * (sv + 1) - b],
                                        v_aug[h][:, tt, :],
                                        start=(i == 0), stop=(i == len(ts) - 1))
                            rec = rrp.tile([128, 2], dt.float32, tag="rec")
                            nc.vector.reciprocal(rec[:], ops[:, :, 64:65])
                            tmp_o = obp.tile([128, 64], dt.float32, tag="tmp_o",
                                             name="tmp_o")
                            nc.vector.tensor_scalar(
                                tmp_o[:], ops[:, 0, 0:64], rec[:, 0:1], None, MUL)
                            rl = rrp.tile([128, 1], dt.float32, tag="rl")
                            nc.vector.tensor_tensor(rl[:], rec[:, 1:2],
                                                    lamn[:, h:h + 1], MUL)
                            pi, sl = sv // 2, sv % 2
                            if pi not in pair_tiles:
                                pair_tiles[pi] = obp.tile(
                                    [128, 2, 64], dt.float16, tag="ob2",
                                    name="ob2", bufs=2)
                                pair_count[pi] = 0
                            ob = pair_tiles[pi][:, sl, :]
                            nc.vector.scalar_tensor_tensor(
                                ob, ops[:, 1, 0:64], rl[:],
                                tmp_o[:], MUL, ADD,
                                accum_out=ssum[:, h, sv:sv + 1])
                            sq = obp.tile([128, 64], dt.float16, tag="sqt",
                                          name="sqt")
                            nc.vector.scalar_tensor_tensor(
                                sq[:], ob, 1.0, ob,
                                mybir.AluOpType.bypass, MUL,
                                accum_out=ssq[:, h, sv:sv + 1])
                            pair_count[pi] += 1
                            if pair_count[pi] == 2:
                                flush_pair(pi)

                    def emit_deferred(slot):
                        # head-1's q/k projections, interleaved into head-0's
                        # attention so phase 1 only computes head-0 operands
                        if h != 0 or slot >= 8:
                            return
                        j, sc = 2 + slot // 4, slot % 4
                        pp = sc_ps.tile([128, 2, 512], dt.float32,
                                        tag="scps", name="pchain")
                        for e in range(8):
                            nc.tensor.matmul(pp[:, 0, :], wqk[:, j, e, :],
                                             xt[:, e, 512 * sc:512 * (sc + 1)],
                                             start=(e == 0), stop=(e == 7))
                        nc.vector.tensor_scalar(
                            qk_sb[j][:, 512 * sc:512 * (sc + 1)],
                            pp[:, 0, :], bqk[:, j:j + 1], None, ADD)

                    for t in range(NT):
                        if s0[t] >= NT:
                            emit_deferred(t)
                            emit_ochains(t - 1)
                            continue
                        base = 128 * int(s0[t])
                        et2 = expp.tile([128, 2, S - base], dt.float16,
                                        tag=f"exp{t}", name=f"exp{t}",
                                        bufs=2 if S - base <= 768 else 1)
                        for m in range(2):
                            exp_tm[m][t] = et2[:, m, :]
                        for pos in range(base, S, 512):
                            w = min(512, S - pos)
                            pss = sc_ps.tile([128, 2, 512], dt.float32,
                                             tag="scps", name="scps")
                            for m in range(2):
                                p0, p1 = 64 * m, 64 * (m + 1)
                                nc.tensor.matmul(
                                    pss[:, m, 0:w],
                                    kT[p0:p1, 128 * t:128 * (t + 1)],
                                    qT[p0:p1, pos:pos + w],
                                    start=True, stop=True)
                            nc.scalar.activation(
                                et2[:, :, pos - base:pos - base + w],
                                pss[:, :, 0:w], Exp, scale=0.125)
                        for sblk in range(int(s0[t]), NT):
                            ki = int(tile_idx[t, sblk])
                            if ki >= 0:
                                off = 128 * sblk - base
                                nc.vector.tensor_tensor(
                                    et2[:, :, off:off + 128],
                                    et2[:, :, off:off + 128],
                                    keep16[:, ki, :, :], MUL)
                        emit_deferred(t)
                        emit_ochains(t - 1)
                    emit_ochains(NT - 1)
                    for pi in sorted(pair_tiles):
                        if pair_count[pi] == 1:  # unpaired sv (general masks)
                            flush_pair(pi)
                    # head finished: stats -> GN affine -> a2a
                    nc.vector.tensor_reduce(stat4[:, 2 * h:2 * h + 1],
                                            ssum[:, h, :], mybir.AxisListType.X,
                                            ADD)
                    nc.vector.tensor_reduce(stat4[:, 2 * h + 1:2 * h + 2],
                                            ssq[:, h, :], mybir.AxisListType.X,
                                            ADD)
                    red = o_ps.tile([1, 2], dt.float32, tag="ops", name="red")
                    nc.tensor.matmul(red[:], ones_col[:], stat4[:, 2 * h:2 * h + 2],
                                     start=True, stop=True)
                    scal = rrp.tile([1, 7], dt.float32, tag="scal")
                    n_inv = 1.0 / (S * DH)
                    # cols: mean, E[x^2], mean^2, var, std, inv, negmu
                    nc.vector.tensor_scalar(scal[:, 0:2], red[:], n_inv, None, MUL)
                    nc.vector.tensor_tensor(scal[:, 2:3], scal[:, 0:1],
                                            scal[:, 0:1], MUL)
                    nc.vector.tensor_tensor(scal[:, 3:4], scal[:, 1:2],
                                            scal[:, 2:3], SUB)
                    nc.scalar.activation(scal[:, 4:5], scal[:, 3:4],
                                         mybir.ActivationFunctionType.Ln,
                                         bias=epsc[0:1, 0:1])
                    nc.scalar.activation(scal[:, 5:6], scal[:, 4:5], Exp,
                                         scale=-0.5)
                    nc.vector.tensor_scalar(scal[:, 6:7], scal[:, 0:1], -1.0,
                                            None, MUL)
                    bc = o_ps.tile([128, 2], dt.float32, tag="ops", name="bc")
                    nc.tensor.matmul(bc[:], ones128[:], scal[:, 5:7],
                                     start=True, stop=True)
                    a_h = rrp.tile([128, 1], dt.float32, tag="a_h")
                    nc.vector.tensor_tensor(a_h[:], gnw2[:, h:h + 1], bc[:, 0:1],
                                            MUL)
                    b_h = rrp.tile([128, 1], dt.float32, tag="b_h")
                    nc.vector.scalar_tensor_tensor(b_h[:], a_h[:], bc[:, 1:2],
                                                   gnb2[:, h:h + 1], MUL, ADD)
                    # GN affine on the pair layout, then a2a_in writes: slice
                    # i's cols 0:128 come from even-sv partitions, 128:256
                    # from odd-sv partitions
                    for i in range(2):
                        nc.vector.tensor_scalar(
                            xn_h[h][:, 4 * i:4 * (i + 1), :],
                            oT2[:, 4 * i:4 * (i + 1), :],
                            a_h[:], b_h[:], MUL, ADD)
                    for hh in range(2):
                        eng = nc.sync if hh == 0 else nc.gpsimd
                        eng.dma_start(
                            out=a2a_in[h][:, 128 * hh:128 * (hh + 1)].rearrange(
                                "(i p) s -> p i s", i=8),
                            in_=xn_h[h][64 * hh:64 * (hh + 1), :, :])
                    nc.gpsimd.collective_compute(
                        "AllToAll", mybir.AluOpType.bypass, replica_groups=groups,
                        ins=[a2a_in[h][:]], outs=[a2a_out[h][:]])

            # ============ phase 4: output Linear on exchanged slices ==========
            # head-0 half runs during the head-1 AllToAll window (it only
            # depends on xa[0:64]); head-1 half accumulates on the same PSUM.
            with tc.tile_pool(name="fin", bufs=1) as finp:
                out_sb = finp.tile([128, 2, E], dt.float32, tag="out_sb")
                # xa loads per head (4 parallel queues each): even j ->
                # partitions 0:64, odd j -> 64:128, grouped by j-pair
                for hh in range(HLOC):
                    for q in range(2):
                        eng = nc.sync if q == 0 else nc.scalar
                        eng.dma_start(
                            out=xa[hh][:, 2 * q:2 * (q + 1), :],
                            in_=a2a_out[hh][256 * q:256 * (q + 1), :].rearrange(
                                "(jp w p) s -> (w p) jp s", jp=2, w=2))
                with tc.tile_pool(name="f_ps", bufs=4, space="PSUM") as f_ps, \
                     tc.tile_pool(name="w_ps", bufs=1, space="PSUM") as w_ps:
                    pss = []
                    for sh in range(2):   # two 128-row halves of the 256-row slice
                        for ec in range(2):  # two 512-col chunks of E
                            ps = f_ps.tile([128, 512], dt.float32, tag="f_ps")
                            pss.append(ps)
                            for jp in range(4):  # head-0 half: runs in the
                                nc.tensor.matmul(  # head-1 AllToAll window
                                    ps[:], xa[0][:, jp, 128 * sh:128 * (sh + 1)],
                                    wo[:, 0, jp, 512 * ec:512 * (ec + 1)],
                                    start=(jp == 0), stop=False)
                    # keep the PE warm (HAM at 8/8) through the remainder of
                    # the AllToAll window with throwaway matmuls
                    wps = w_ps.tile([128, 256], dt.float32, tag="w_ps")
                    for i in range(40):
                        nc.tensor.matmul(wps[:], wo[:, 0, 0, 0:128],
                                         wo[:, 0, 1, 0:256],
                                         start=True, stop=True)
                    for sh in range(2):
                        for ec in range(2):
                            ps = pss[2 * sh + ec]
                            for jp in range(4):
                                nc.tensor.matmul(
                                    ps[:], xa[1][:, jp, 128 * sh:128 * (sh + 1)],
                                    wo[:, 1, jp, 512 * ec:512 * (ec + 1)],
                                    start=False, stop=(jp == 3))
                            nc.vector.tensor_tensor(
                                out_sb[:, sh, 512 * ec:512 * (ec + 1)], ps[:],
                                bo_bc[:, 512 * ec:512 * (ec + 1)], ADD)
                        nc.sync.dma_start(out=OUT[128 * sh:128 * (sh + 1), :],
                                          in_=out_sb[:, sh, :])
    nc.finalize()
    return nc


def _prep_inputs(x, mask, Wq1, bq1, Wq2, bq2, Wk1, bk1, Wk2, bk2, Wv, bv,
                 lam, gn_w, gn_b, Wo, bo):
    f32 = np.float32
    f16 = np.float16
    x = np.asarray(x, f32).reshape(S, E)
    mask = np.asarray(mask, bool)
    state, s0, tile_idx, keep = _classify_mask(mask)
    key = (state.tobytes(), s0.tobytes())
    if key not in _PROGRAM_CACHE:
        _PROGRAM_CACHE[key] = _build_program(state, s0, tile_idx, len(keep))
    nc = _PROGRAM_CACHE[key]

    # [E, S] -> partition-major [128, 8*S]
    xT = np.ascontiguousarray(
        x.T.reshape(8, 128, S).transpose(1, 0, 2).reshape(128, 8 * S)).astype(f16)
    # Wo^T rows in the (j-pair, head) layout matching xa:
    # wo[64*(j%2)+p, h, j//2, :] = WoT[128*j + 64*h + p, :]
    WoT = np.asarray(Wo, f32).T  # [D, E]
    woT = np.zeros((128, HLOC, 4, E), f32)
    for j in range(NCORES):
        for h in range(HLOC):
            woT[64 * (j % 2):64 * (j % 2) + 64, h, j // 2, :] = \
                WoT[128 * j + 64 * h:128 * j + 64 * h + 64, :]
    woT = np.ascontiguousarray(woT.reshape(128, 8 * E)).astype(f16)
    eye = np.eye(128, dtype=f32).astype(f16)
    b16 = np.concatenate([np.ones((1, 128), f32),
                          np.asarray(bo, f32).reshape(1, E)], axis=1).astype(f16)
    # keep tiles duplicated along a 2-wide m axis: [128, nk, 2, 128]
    keep_p = np.ascontiguousarray(
        np.repeat(keep.transpose(1, 0, 2)[:, :, None, :], 2, axis=2)
        .reshape(128, -1))

    Wq1, Wq2, Wk1, Wk2, Wv = (np.asarray(a, f32) for a in (Wq1, Wq2, Wk1, Wk2, Wv))
    bq1, bq2, bk1, bk2, bv = (np.asarray(a, f32) for a in (bq1, bq2, bk1, bk2, bv))
    lam = np.asarray(lam, f32)
    gn_w = np.asarray(gn_w, f32)
    gn_b = np.asarray(gn_b, f32)

    in_maps = []
    for c in range(NCORES):
        h0, h1 = 2 * c, 2 * c + 1
        wqk = np.stack([
            np.concatenate([Wq1[h0].T, Wq2[h0].T], axis=1),
            np.concatenate([Wk1[h0].T, Wk2[h0].T], axis=1),
            np.concatenate([Wq1[h1].T, Wq2[h1].T], axis=1),
            np.concatenate([Wk1[h1].T, Wk2[h1].T], axis=1)])  # [4, 1024, 128]
        wqk_p = np.ascontiguousarray(
            wqk.reshape(4, 8, 128, 128).transpose(2, 0, 1, 3)
            .reshape(128, -1)).astype(f16)
        wv = np.concatenate([Wv[h0].T, Wv[h1].T], axis=1)  # [1024, 128]
        wv_p = np.ascontiguousarray(
            wv.reshape(8, 128, 128).transpose(1, 0, 2).reshape(128, -1)).astype(f16)
        blob = np.zeros((128, 11), f32)
        blob[:, 0] = np.concatenate([bq1[h0], bq2[h0]])
        blob[:, 1] = np.concatenate([bk1[h0], bk2[h0]])
        blob[:, 2] = np.concatenate([bq1[h1], bq2[h1]])
        blob[:, 3] = np.concatenate([bk1[h1], bk2[h1]])
        blob[:, 4] = np.concatenate([bv[h0], bv[h1]])
        blob[:, 5] = -lam[h0]
        blob[:, 6] = -lam[h1]
        blob[0:64, 7] = blob[64:128, 7] = gn_w[DH * h0:DH * (h0 + 1)]
        blob[0:64, 8] = blob[64:128, 8] = gn_w[DH * h1:DH * (h1 + 1)]
        blob[0:64, 9] = blob[64:128, 9] = gn_b[DH * h0:DH * (h0 + 1)]
        blob[0:64, 10] = blob[64:128, 10] = gn_b[DH * h1:DH * (h1 + 1)]
        in_maps.append({
            "xt": xT, "wqk": wqk_p, "wv": wv_p, "blob": blob,
            "keep": keep_p, "eye": eye, "wo": woT, "b16": b16,
        })
    return nc, in_maps


def kernel(**inputs):
    import time
    nc, in_maps = _prep_inputs(**inputs)
    last = None
    for attempt in range(3):
        try:
            res = run_bass_kernel_spmd(nc, in_maps, list(range(NCORES)))
            break
        except Exception as e:  # transient device hiccups: retry
            last = e
            time.sleep(10 * (attempt + 1))
    else:
        raise last
    out = np.concatenate([res.results[c]["out_slice"] for c in range(NCORES)],
                         axis=0)
    return out.reshape(1, S, E).astype(np.float32)
